# revision 9
# baseline (speedup 1.0000x reference)
"""GraphSAGE 3-layer GNN on 8 TRN2 NeuronCores.

Strategy (node-sharded, feature-replicated):
  - 50000 nodes padded to 50176 = 8 cores x 49 blocks x 128. Core c owns
    destination rows [6272c, 6272c+6272).
  - Per layer, every core gathers h[src] rows for its local edges with
    gpsimd.dma_gather (one 512B/256B row per edge, edge lands on one SBUF
    partition), then segment-sums them per 128-dst block with a one-hot
    (x 1/deg) matmul on TensorE:  meanT[feat,dst] += G[e,feat]^T.T @ S[e,dst].
  - Dense SAGE terms run in transposed orientation so tanh bias is
    per-partition:  outT[feat_out, node] = Wl @ meanT + Wr @ hT (+bias).
  - Dropout masks are precomputed on the host as {0, 1.25} multipliers.
  - Between layers the node-major output shard is AllGather'd so every core
    can gather any source row next layer. Layer 3 aggregates the
    W3l-projected features p = h2 @ W3l^T (128 dims), so only p is gathered
    (halves layer-3 traffic); h2 stays local for the W3r term.
  - int16 gather indices can't span 50176 rows, so the node space is split
    at HALF: per (block, half) edge chunks gather from the lo/hi view.

kernel(**inputs) takes full-size numpy inputs, does all index/layout prep on
the host, compiles one SPMD Bass program, runs it on cores 0-7, and returns
the full [50000, 128] float32 output.
"""

import math
from contextlib import ExitStack

import ml_dtypes
import numpy as np

import concourse.bass as bass
import concourse.bacc as bacc
import concourse.mybir as mybir
import concourse.tile as tile
from concourse.bass_utils import run_bass_kernel_spmd
from concourse.masks import make_identity

P = 128
NCORES = 8
DROP_P = 0.2
BF16 = ml_dtypes.bfloat16

F32 = mybir.dt.float32
BF16_T = mybir.dt.bfloat16
I16 = mybir.dt.int16


# ----------------------------------------------------------------------------
# Planning (shared across cores; the SPMD program structure depends on it)
# ----------------------------------------------------------------------------

class Plan:
    pass


def make_plan(n_nodes, n_edges, d_in, d_out, edge_index, gmax=22):
    pl = Plan()
    pl.N = n_nodes
    pl.E = n_edges
    pl.D = d_in
    pl.DO = d_out
    pl.KB = d_in // P
    assert d_in % P == 0 and d_out == P
    pl.NB = math.ceil(n_nodes / NCORES / P)     # blocks per core
    pl.NPC = pl.NB * P                          # padded nodes per core
    pl.NPAD = NCORES * pl.NPC

    src = np.asarray(edge_index[0], dtype=np.int64)
    dst = np.asarray(edge_index[1], dtype=np.int64)
    assert src.min() >= 0 and src.max() < n_nodes
    deg = np.bincount(dst, minlength=n_nodes)
    pl.invdeg = (1.0 / np.maximum(deg, 1)).astype(np.float32)

    core = dst // pl.NPC
    lb = (dst - core * pl.NPC) // P             # block within core
    pl.dst_local = (dst - core * pl.NPC) % P
    pl.core = core
    pl.lb = lb
    pl.src = src
    pl.dst = dst

    # pick HALF (node-id split so gather indices fit int16) minimizing chunks
    lo_min = max(0, pl.NPAD - 32768)
    candidates = list(range(max(1024, lo_min), min(32768, pl.NPAD) + 1, 1024))
    if pl.NPAD <= 32768:
        candidates = [pl.NPAD]  # no split needed
    best = None
    key_base = (core * pl.NB + lb) * 2
    for half in candidates:
        is_hi = (src >= half).astype(np.int64)
        cnt = np.bincount(key_base + is_hi, minlength=NCORES * pl.NB * 2)
        cnt = cnt.reshape(NCORES, pl.NB, 2)
        nchunks = -(-cnt.max(axis=0) // P)      # [NB, 2] ceil
        tot = int(nchunks.sum())
        if best is None or tot < best[0]:
            best = (tot, half, nchunks)
    pl.total_chunks_min, pl.HALF, nch = best
    pl.nlo = nch[:, 0].astype(int)
    pl.nhi = nch[:, 1].astype(int)

    # supergroups: consecutive blocks with per-half chunk sums <= gmax
    sgs = []
    cur, slo, shi = [], 0, 0
    for b in range(pl.NB):
        if cur and (slo + pl.nlo[b] > gmax or shi + pl.nhi[b] > gmax):
            sgs.append(cur)
            cur, slo, shi = [], 0, 0
        cur.append(b)
        slo += pl.nlo[b]
        shi += pl.nhi[b]
    if cur:
        sgs.append(cur)

    # global chunk ordering: per sg -> lo chunks (block order) -> hi chunks
    pl.sgs = []
    cg = 0
    pl.block_chunks = [None] * pl.NB   # block -> list of (sg_idx, half, local_slot, cg)
    for b in range(pl.NB):
        pl.block_chunks[b] = []
    for si, blocks in enumerate(sgs):
        info = {"blocks": blocks}
        for half, narr in (("lo", pl.nlo), ("hi", pl.nhi)):
            start = cg
            for b in blocks:
                for j in range(narr[b]):
                    pl.block_chunks[b].append((si, half, cg - start, cg))
                    cg += 1
            info[half + "_start"] = start
            info[half + "_n"] = cg - start
        pl.sgs.append(info)
    pl.NCHUNK = cg
    pl.GMAX = max(
        max((sg["lo_n"] for sg in pl.sgs), default=1),
        max((sg["hi_n"] for sg in pl.sgs), default=1),
        1,
    )
    return pl


# ----------------------------------------------------------------------------
# Per-core host packing
# ----------------------------------------------------------------------------

def _featT(arr, KB):
    """[n, KB*128] -> [128, KB, n]  (feature-major layout)."""
    n = arr.shape[0]
    return np.ascontiguousarray(
        arr.T.reshape(KB, P, n).transpose(1, 0, 2)
    )


def pack_core(pl, c, x_pad_bf16, scale1, scale2):
    """Build S, idx, xT, maskT arrays for core c."""
    NB, NPC, NCHUNK = pl.NB, pl.NPC, pl.NCHUNK
    mine = pl.core == c
    src, lb, dl = pl.src[mine], pl.lb[mine], pl.dst_local[mine]
    w = pl.invdeg[pl.dst[mine]]
    is_hi = src >= pl.HALF

    # slot assignment: order edges by (lb, half); chunk starts from plan
    order = np.lexsort((is_hi, lb))
    src, lb, dl, w, is_hi = (a[order] for a in (src, lb, dl, w, is_hi))

    # chunk start (in global cg units) per (block, half)
    cg_start = np.zeros((NB, 2), dtype=np.int64)
    for b in range(NB):
        lo_list = [cg for (_, h, _, cg) in pl.block_chunks[b] if h == "lo"]
        hi_list = [cg for (_, h, _, cg) in pl.block_chunks[b] if h == "hi"]
        cg_start[b, 0] = lo_list[0] if lo_list else 0
        cg_start[b, 1] = hi_list[0] if hi_list else 0

    # rank within (block, half) group
    key = lb * 2 + is_hi
    grp_first = np.searchsorted(key, np.arange(NB * 2))
    rank = np.arange(len(src)) - grp_first[key]
    pos = cg_start[lb, is_hi.astype(int)] * P + rank           # global slot
    cap = None  # sanity: rank must fit the planned chunks
    nch = np.stack([pl.nlo, pl.nhi], axis=1)
    assert (rank < nch[lb, is_hi.astype(int)] * P).all(), "chunk overflow"

    idx_vals = np.zeros(NCHUNK * P, dtype=np.int16)
    idx_vals[pos] = np.where(is_hi, src - pl.HALF, src).astype(np.int16)

    S = np.zeros((P, NCHUNK, P), dtype=BF16)
    S[pos % P, pos // P, dl] = w.astype(BF16)

    # wrap idx into [128, NCHUNK*8] int16, per-gather-call 16-element wrap
    idx_all = np.zeros((P, NCHUNK * 8), dtype=np.int16)
    for sg in pl.sgs:
        for half in ("lo", "hi"):
            s0, n = sg[half + "_start"], sg[half + "_n"]
            if n == 0:
                continue
            seg = idx_vals[s0 * P:(s0 + n) * P]
            blk = seg.reshape(-1, 16).T                        # [16, n*8]
            idx_all[:, s0 * 8:(s0 + n) * 8] = np.tile(blk, (8, 1))

    lo_node, hi_node = c * NPC, min((c + 1) * NPC, pl.N)
    nreal = hi_node - lo_node
    xo = np.zeros((NPC, pl.D), dtype=BF16)
    xo[:nreal] = x_pad_bf16[lo_node:hi_node]
    m1 = np.zeros((NPC, pl.D), dtype=np.float32)
    m2 = np.zeros((NPC, pl.D), dtype=np.float32)
    m1[:nreal] = scale1[lo_node:hi_node]
    m2[:nreal] = scale2[lo_node:hi_node]

    return {
        "s_mat": S,
        "idx_all": idx_all,
        "xT": _featT(xo, pl.KB).astype(BF16),
        "m1T": _featT(m1, pl.KB).astype(BF16),
        "m2T": _featT(m2, pl.KB).astype(BF16),
    }


# ----------------------------------------------------------------------------
# Bass program
# ----------------------------------------------------------------------------

def build_program(pl, n_cores=NCORES, g_bufs=3):
    D, DO, KB, NB, NPC, NPAD = pl.D, pl.DO, pl.KB, pl.NB, pl.NPC, pl.NPAD
    NCHUNK, GMAX, HALF = pl.NCHUNK, pl.GMAX, pl.HALF
    Tanh = mybir.ActivationFunctionType.Tanh

    nc = bacc.Bacc("TRN2", target_bir_lowering=False, debug=False,
                   num_devices=n_cores, dynamic_dma_scratch_size=16384)

    # ---- DRAM parameters
    x_rows = nc.dram_tensor("x_rows", [NPAD, D], BF16_T, kind="ExternalInput")
    xT_d = nc.dram_tensor("xT", [P, KB, NPC], BF16_T, kind="ExternalInput")
    m1_d = nc.dram_tensor("m1T", [P, KB, NPC], BF16_T, kind="ExternalInput")
    m2_d = nc.dram_tensor("m2T", [P, KB, NPC], BF16_T, kind="ExternalInput")
    s_d = nc.dram_tensor("s_mat", [P, NCHUNK, P], BF16_T, kind="ExternalInput")
    idx_d = nc.dram_tensor("idx_all", [P, NCHUNK * 8], I16, kind="ExternalInput")
    w1l_d = nc.dram_tensor("w1l", [P, KB, D], BF16_T, kind="ExternalInput")
    w1r_d = nc.dram_tensor("w1r", [P, KB, D], BF16_T, kind="ExternalInput")
    w2l_d = nc.dram_tensor("w2l", [P, KB, D], BF16_T, kind="ExternalInput")
    w2r_d = nc.dram_tensor("w2r", [P, KB, D], BF16_T, kind="ExternalInput")
    w3l_d = nc.dram_tensor("w3l", [P, KB, DO], BF16_T, kind="ExternalInput")
    w3r_d = nc.dram_tensor("w3r", [P, KB, DO], BF16_T, kind="ExternalInput")
    bias_d = nc.dram_tensor("bias", [P, 2 * KB + 1], F32, kind="ExternalInput")
    out_d = nc.dram_tensor("out", [NPC, DO], F32, kind="ExternalOutput")

    rg = [list(range(n_cores))]

    with tile.TileContext(nc) as tc, ExitStack() as ctx:
        consts = ctx.enter_context(tc.tile_pool(name="consts", bufs=1))
        gpool = ctx.enter_context(tc.tile_pool(name="gpool", bufs=g_bufs))
        work = ctx.enter_context(tc.tile_pool(name="work", bufs=3))
        dram = ctx.enter_context(tc.tile_pool(name="dram", bufs=1, space="DRAM"))
        psA = ctx.enter_context(tc.tile_pool(name="psA", bufs=2, space="PSUM"))
        psB = ctx.enter_context(tc.tile_pool(name="psB", bufs=2, space="PSUM"))
        psT = ctx.enter_context(tc.tile_pool(name="psT", bufs=2, space="PSUM"))
        psP = ctx.enter_context(tc.tile_pool(name="psP", bufs=2, space="PSUM"))

        # ---- internal DRAM (collective bounce + gathered features)
        cc1_in = dram.tile([NPC, D], BF16_T)
        h1_full = dram.tile([NPAD, D], BF16_T, addr_space="Shared")
        ccp_in = dram.tile([NPC, DO], BF16_T)
        p_full = dram.tile([NPAD, DO], BF16_T, addr_space="Shared")

        # ---- resident SBUF
        s_sb = consts.tile([P, NCHUNK, P], BF16_T)
        nc.sync.dma_start(s_sb[:], s_d[:])
        w1l = consts.tile([P, KB, D], BF16_T)
        nc.sync.dma_start(w1l[:], w1l_d[:])
        w1r = consts.tile([P, KB, D], BF16_T)
        nc.sync.dma_start(w1r[:], w1r_d[:])
        w2l = consts.tile([P, KB, D], BF16_T)
        nc.sync.dma_start(w2l[:], w2l_d[:])
        w2r = consts.tile([P, KB, D], BF16_T)
        nc.sync.dma_start(w2r[:], w2r_d[:])
        w3l = consts.tile([P, KB, DO], BF16_T)
        nc.sync.dma_start(w3l[:], w3l_d[:])
        w3r = consts.tile([P, KB, DO], BF16_T)
        nc.sync.dma_start(w3r[:], w3r_d[:])
        bias_sb = consts.tile([P, 2 * KB + 1], F32)
        nc.sync.dma_start(bias_sb[:], bias_d[:])
        ident_b = consts.tile([P, P], BF16_T)
        make_identity(nc, ident_b)
        ident_f = consts.tile([P, P], F32)
        make_identity(nc, ident_f)

        # per-block resident hT tiles (feature-major current features)
        ht = []
        for b in range(NB):
            t = consts.tile([P, KB, P], BF16_T, name=f"ht{b}", tag=f"ht{b}")
            nc.sync.dma_start(t[:], xT_d[:, :, b * P:(b + 1) * P])
            ht.append(t)

        layers = [
            dict(wl=w1l, wr=w1r, mask=m1_d, bcol=0, src=x_rows, elem=D),
            dict(wl=w2l, wr=w2r, mask=m2_d, bcol=KB, src=h1_full, elem=D),
            dict(wr3=w3r, bcol=2 * KB, src=p_full, elem=DO),
        ]

        for li, L in enumerate(layers):
            src_t = L["src"]
            elem = L["elem"]
            src_lo = src_t[0:HALF, :]
            src_hi = src_t[HALF:NPAD, :]
            last3 = li == 2

            for si, sg in enumerate(pl.sgs):
                # gathers for this supergroup
                tiles = {}
                for half, sv in (("lo", src_lo), ("hi", src_hi)):
                    n = sg[half + "_n"]
                    if n == 0:
                        tiles[half] = None
                    else:
                        s0 = sg[half + "_start"]
                        idx_t = work.tile([P, GMAX * 8], I16, tag="idx")
                        nc.sync.dma_start(
                            idx_t[:, :n * 8], idx_d[:, s0 * 8:(s0 + n) * 8])
                        g_t = gpool.tile([P, GMAX, D], BF16_T, tag="g")
                        if last3:
                            gv = g_t.rearrange("p g (a b) -> p (g a) b", b=DO)
                        else:
                            gv = g_t
                        nc.gpsimd.dma_gather(
                            gv[:, :n, :elem], sv, idx_t[:, :n * 8],
                            n * P, n * P, elem, single_packet=False)
                        tiles[half] = gv

                for b in sg["blocks"]:
                    bsl = slice(b * P, (b + 1) * P)
                    chunks = pl.block_chunks[b]  # (sg_idx, half, slot, cg)
                    my = [(tiles[h], sl, cg) for (s, h, sl, cg) in chunks
                          if s == si]
                    nch = len(my)

                    if not last3:
                        # segment mean (transposed): meanT[feat,dst]
                        mps = psA.tile([P, KB, P], F32, tag="acc")
                        for k in range(KB):
                            for ci, (gt, sl, cg) in enumerate(my):
                                nc.tensor.matmul(
                                    mps[:, k, :],
                                    gt[:, sl, k * P:(k + 1) * P],
                                    s_sb[:, cg, :],
                                    start=(ci == 0), stop=(ci == nch - 1))
                        m_sb = work.tile([P, KB, P], BF16_T, tag="msb")
                        if nch == 0:
                            nc.vector.memset(m_sb[:], 0.0)
                        else:
                            nc.vector.tensor_copy(m_sb[:], mps[:])

                        # dense: outT[feat_out, node] = Wl@meanT + Wr@hT
                        ops = psB.tile([P, KB, P], F32, tag="out")
                        for bank in range(KB):
                            for k in range(KB):
                                nc.tensor.matmul(
                                    ops[:, bank, :],
                                    L["wl"][:, k, bank * P:(bank + 1) * P],
                                    m_sb[:, k, :],
                                    start=(k == 0), stop=False)
                            for k in range(KB):
                                nc.tensor.matmul(
                                    ops[:, bank, :],
                                    L["wr"][:, k, bank * P:(bank + 1) * P],
                                    ht[b][:, k, :],
                                    start=False, stop=(k == KB - 1))

                        # epilogue: tanh(+bias), dropout mask, update hT
                        mk_t = work.tile([P, KB, P], BF16_T, tag="mk")
                        nc.sync.dma_start(mk_t[:], L["mask"][:, :, bsl])
                        a_sb = work.tile([P, KB, P], BF16_T, tag="act")
                        for bank in range(KB):
                            nc.scalar.activation(
                                a_sb[:, bank, :], ops[:, bank, :], Tanh,
                                bias=bias_sb[:, L["bcol"] + bank:L["bcol"] + bank + 1])
                        nc.vector.tensor_mul(
                            out=ht[b][:], in0=a_sb[:], in1=mk_t[:])

                        if li == 0:
                            # node-major copy for AllGather input
                            nm = work.tile([P, D], BF16_T, tag="nm")
                            for bank in range(KB):
                                tp = psT.tile([P, P], BF16_T, tag="tp")
                                nc.tensor.transpose(
                                    tp, ht[b][:, bank, :], ident_b)
                                nc.vector.tensor_copy(
                                    nm[:, bank * P:(bank + 1) * P], tp)
                            nc.sync.dma_start(cc1_in[bsl, :], nm)

                        if li == 1:
                            # p = h2 @ W3l^T  (node-major), for layer-3 gather
                            pp = psP.tile([P, DO], F32, tag="pp")
                            for k in range(KB):
                                nc.tensor.matmul(
                                    pp, ht[b][:, k, :], w3l[:, k, :],
                                    start=(k == 0), stop=(k == KB - 1))
                            p_sb = work.tile([P, DO], BF16_T, tag="pnm")
                            nc.vector.tensor_copy(p_sb, pp)
                            nc.sync.dma_start(ccp_in[bsl, :], p_sb)
                    else:
                        # layer 3: outT = mean(p)^T + W3r @ hT, tanh, output
                        ops = psB.tile([P, KB, P], F32, tag="out")
                        o3 = ops[:, 0, :]
                        for ci, (gt, sl, cg) in enumerate(my):
                            nc.tensor.matmul(
                                o3, gt[:, sl, :], s_sb[:, cg, :],
                                start=(ci == 0), stop=False)
                        for k in range(KB):
                            nc.tensor.matmul(
                                o3, L["wr3"][:, k, :], ht[b][:, k, :],
                                start=(nch == 0 and k == 0), stop=(k == KB - 1))
                        o_sb = work.tile([P, DO], F32, tag="o3")
                        nc.scalar.activation(
                            o_sb, o3, Tanh,
                            bias=bias_sb[:, L["bcol"]:L["bcol"] + 1])
                        tpf = psP.tile([P, DO], F32, tag="pp")
                        nc.tensor.transpose(tpf, o_sb, ident_f)
                        onm = work.tile([P, DO], F32, tag="onm")
                        nc.vector.tensor_copy(onm, tpf)
                        nc.sync.dma_start(out_d[bsl, :], onm)

            # collectives at layer boundaries
            if li == 0:
                nc.gpsimd.collective_compute(
                    "AllGather", mybir.AluOpType.bypass,
                    replica_groups=rg,
                    ins=[cc1_in.opt()], outs=[h1_full.opt()])
            elif li == 1:
                nc.gpsimd.collective_compute(
                    "AllGather", mybir.AluOpType.bypass,
                    replica_groups=rg,
                    ins=[ccp_in.opt()], outs=[p_full.opt()])

    nc.compile()
    return nc


# ----------------------------------------------------------------------------
# Host driver
# ----------------------------------------------------------------------------

def prepare(x, edge_index, mask1, mask2,
            W1l, b1, W1r, W2l, b2, W2r, W3l, b3, W3r, gmax=22):
    N, D = x.shape
    DO = W3l.shape[0]
    E = edge_index.shape[1]
    pl = make_plan(N, E, D, DO, edge_index, gmax=gmax)
    KB = pl.KB

    x_bf = x.astype(BF16)
    x_pad = np.zeros((pl.NPAD, D), dtype=BF16)
    x_pad[:N] = x_bf
    scale1 = ((mask1 > DROP_P) / (1.0 - DROP_P)).astype(np.float32)
    scale2 = ((mask2 > DROP_P) / (1.0 - DROP_P)).astype(np.float32)

    def packw(W):
        return np.ascontiguousarray(
            W.T.reshape(KB, P, W.shape[0]).transpose(1, 0, 2)).astype(BF16)

    bias = np.zeros((P, 2 * KB + 1), dtype=np.float32)
    for k in range(KB):
        bias[:, k] = b1[k * P:(k + 1) * P]
        bias[:, KB + k] = b2[k * P:(k + 1) * P]
    bias[:, 2 * KB] = b3[:P]

    shared = {
        "x_rows": x_pad,
        "w1l": packw(W1l), "w1r": packw(W1r),
        "w2l": packw(W2l), "w2r": packw(W2r),
        "w3l": packw(W3l), "w3r": packw(W3r),
        "bias": bias,
    }
    in_maps = []
    for c in range(NCORES):
        m = dict(shared)
        m.update(pack_core(pl, c, x_pad, scale1, scale2))
        in_maps.append(m)
    return pl, in_maps


_CACHE = {}


def kernel(x, edge_index, mask1, mask2,
           W1l, b1, W1r, W2l, b2, W2r, W3l, b3, W3r):
    x = np.asarray(x, dtype=np.float32)
    args = (np.asarray(edge_index), np.asarray(mask1, dtype=np.float32),
            np.asarray(mask2, dtype=np.float32))
    pl, in_maps = prepare(
        x, args[0], args[1], args[2],
        np.asarray(W1l, np.float32), np.asarray(b1, np.float32),
        np.asarray(W1r, np.float32),
        np.asarray(W2l, np.float32), np.asarray(b2, np.float32),
        np.asarray(W2r, np.float32),
        np.asarray(W3l, np.float32), np.asarray(b3, np.float32),
        np.asarray(W3r, np.float32))
    nc = build_program(pl)
    res = run_bass_kernel_spmd(nc, in_maps, core_ids=list(range(NCORES)))
    N = x.shape[0]
    out = np.zeros((N, pl.DO), dtype=np.float32)
    for c in range(NCORES):
        lo, hi = c * pl.NPC, min((c + 1) * pl.NPC, N)
        out[lo:hi] = res.results[c]["out"][:hi - lo]
    return out


# revision 10
# speedup vs baseline: 1.0356x; 1.0356x over previous
"""GraphSAGE 3-layer GNN on 8 TRN2 NeuronCores.

Strategy (node-sharded, feature-replicated):
  - 50000 nodes padded to 50176 = 8 cores x 49 blocks x 128. Core c owns
    destination rows [6272c, 6272c+6272).
  - Per layer, every core gathers h[src] rows for its local edges with
    gpsimd.dma_gather (one 512B/256B row per edge, edge lands on one SBUF
    partition), then segment-sums them per 128-dst block with a one-hot
    (x 1/deg) matmul on TensorE:  meanT[feat,dst] += G[e,feat]^T.T @ S[e,dst].
  - Dense SAGE terms run in transposed orientation so tanh bias is
    per-partition:  outT[feat_out, node] = Wl @ meanT + Wr @ hT (+bias).
  - Dropout masks are precomputed on the host as {0, 1.25} multipliers.
  - Between layers the node-major output shard is AllGather'd so every core
    can gather any source row next layer. Layer 3 aggregates the
    W3l-projected features p = h2 @ W3l^T (128 dims), so only p is gathered
    (halves layer-3 traffic); h2 stays local for the W3r term.
  - int16 gather indices can't span 50176 rows, so the node space is split
    at HALF: per (block, half) edge chunks gather from the lo/hi view.

kernel(**inputs) takes full-size numpy inputs, does all index/layout prep on
the host, compiles one SPMD Bass program, runs it on cores 0-7, and returns
the full [50000, 128] float32 output.
"""

import math
from contextlib import ExitStack

import ml_dtypes
import numpy as np

import concourse.bass as bass
import concourse.bacc as bacc
import concourse.mybir as mybir
import concourse.tile as tile
from concourse.bass_utils import run_bass_kernel_spmd
from concourse.masks import make_identity

P = 128
NCORES = 8
DROP_P = 0.2
BF16 = ml_dtypes.bfloat16

F32 = mybir.dt.float32
BF16_T = mybir.dt.bfloat16
I16 = mybir.dt.int16


# ----------------------------------------------------------------------------
# Planning (shared across cores; the SPMD program structure depends on it)
# ----------------------------------------------------------------------------

class Plan:
    pass


def make_plan(n_nodes, n_edges, d_in, d_out, edge_index, gmax=22):
    pl = Plan()
    pl.N = n_nodes
    pl.E = n_edges
    pl.D = d_in
    pl.DO = d_out
    pl.KB = d_in // P
    assert d_in % P == 0 and d_out == P
    pl.NB = math.ceil(n_nodes / NCORES / P)     # blocks per core
    pl.NPC = pl.NB * P                          # padded nodes per core
    pl.NPAD = NCORES * pl.NPC

    src = np.asarray(edge_index[0], dtype=np.int64)
    dst = np.asarray(edge_index[1], dtype=np.int64)
    assert src.min() >= 0 and src.max() < n_nodes
    deg = np.bincount(dst, minlength=n_nodes)
    pl.invdeg = (1.0 / np.maximum(deg, 1)).astype(np.float32)

    core = dst // pl.NPC
    lb = (dst - core * pl.NPC) // P             # block within core
    pl.dst_local = (dst - core * pl.NPC) % P
    pl.core = core
    pl.lb = lb
    pl.src = src
    pl.dst = dst

    # pick HALF (node-id split so gather indices fit int16) minimizing chunks
    lo_min = max(0, pl.NPAD - 32768)
    candidates = list(range(max(1024, lo_min), min(32768, pl.NPAD) + 1, 1024))
    if pl.NPAD <= 32768:
        candidates = [pl.NPAD]  # no split needed
    best = None
    key_base = (core * pl.NB + lb) * 2
    for half in candidates:
        is_hi = (src >= half).astype(np.int64)
        cnt = np.bincount(key_base + is_hi, minlength=NCORES * pl.NB * 2)
        cnt = cnt.reshape(NCORES, pl.NB, 2)
        nchunks = -(-cnt.max(axis=0) // P)      # [NB, 2] ceil
        tot = int(nchunks.sum())
        if best is None or tot < best[0]:
            best = (tot, half, nchunks)
    pl.total_chunks_min, pl.HALF, nch = best
    pl.nlo = nch[:, 0].astype(int)
    pl.nhi = nch[:, 1].astype(int)

    # supergroups: consecutive blocks with per-half chunk sums <= gmax
    sgs = []
    cur, slo, shi = [], 0, 0
    for b in range(pl.NB):
        if cur and (slo + pl.nlo[b] > gmax or shi + pl.nhi[b] > gmax):
            sgs.append(cur)
            cur, slo, shi = [], 0, 0
        cur.append(b)
        slo += pl.nlo[b]
        shi += pl.nhi[b]
    if cur:
        sgs.append(cur)

    # global chunk ordering: per sg -> lo chunks (block order) -> hi chunks
    pl.sgs = []
    cg = 0
    pl.block_chunks = [None] * pl.NB   # block -> list of (sg_idx, half, local_slot, cg)
    for b in range(pl.NB):
        pl.block_chunks[b] = []
    for si, blocks in enumerate(sgs):
        info = {"blocks": blocks}
        for half, narr in (("lo", pl.nlo), ("hi", pl.nhi)):
            start = cg
            for b in blocks:
                for j in range(narr[b]):
                    pl.block_chunks[b].append((si, half, cg - start, cg))
                    cg += 1
            info[half + "_start"] = start
            info[half + "_n"] = cg - start
        pl.sgs.append(info)
    pl.NCHUNK = cg
    pl.GMAX = max(
        max((sg["lo_n"] for sg in pl.sgs), default=1),
        max((sg["hi_n"] for sg in pl.sgs), default=1),
        1,
    )
    return pl


# ----------------------------------------------------------------------------
# Per-core host packing
# ----------------------------------------------------------------------------

def _featT(arr, KB):
    """[n, KB*128] -> [128, KB, n]  (feature-major layout)."""
    n = arr.shape[0]
    return np.ascontiguousarray(
        arr.T.reshape(KB, P, n).transpose(1, 0, 2)
    )


def pack_core(pl, c, x_pad_bf16, scale1, scale2):
    """Build S, idx, xT, maskT arrays for core c."""
    NB, NPC, NCHUNK = pl.NB, pl.NPC, pl.NCHUNK
    mine = pl.core == c
    src, lb, dl = pl.src[mine], pl.lb[mine], pl.dst_local[mine]
    w = pl.invdeg[pl.dst[mine]]
    is_hi = src >= pl.HALF

    # slot assignment: order edges by (lb, half); chunk starts from plan
    order = np.lexsort((is_hi, lb))
    src, lb, dl, w, is_hi = (a[order] for a in (src, lb, dl, w, is_hi))

    # chunk start (in global cg units) per (block, half)
    cg_start = np.zeros((NB, 2), dtype=np.int64)
    for b in range(NB):
        lo_list = [cg for (_, h, _, cg) in pl.block_chunks[b] if h == "lo"]
        hi_list = [cg for (_, h, _, cg) in pl.block_chunks[b] if h == "hi"]
        cg_start[b, 0] = lo_list[0] if lo_list else 0
        cg_start[b, 1] = hi_list[0] if hi_list else 0

    # rank within (block, half) group
    key = lb * 2 + is_hi
    grp_first = np.searchsorted(key, np.arange(NB * 2))
    rank = np.arange(len(src)) - grp_first[key]
    pos = cg_start[lb, is_hi.astype(int)] * P + rank           # global slot
    cap = None  # sanity: rank must fit the planned chunks
    nch = np.stack([pl.nlo, pl.nhi], axis=1)
    assert (rank < nch[lb, is_hi.astype(int)] * P).all(), "chunk overflow"

    idx_vals = np.zeros(NCHUNK * P, dtype=np.int16)
    idx_vals[pos] = np.where(is_hi, src - pl.HALF, src).astype(np.int16)

    S = np.zeros((P, NCHUNK, P), dtype=BF16)
    S[pos % P, pos // P, dl] = w.astype(BF16)

    # wrap idx into [128, NCHUNK*8] int16, per-gather-call 16-element wrap
    idx_all = np.zeros((P, NCHUNK * 8), dtype=np.int16)
    for sg in pl.sgs:
        for half in ("lo", "hi"):
            s0, n = sg[half + "_start"], sg[half + "_n"]
            if n == 0:
                continue
            seg = idx_vals[s0 * P:(s0 + n) * P]
            blk = seg.reshape(-1, 16).T                        # [16, n*8]
            idx_all[:, s0 * 8:(s0 + n) * 8] = np.tile(blk, (8, 1))

    lo_node, hi_node = c * NPC, min((c + 1) * NPC, pl.N)
    nreal = hi_node - lo_node
    xo = np.zeros((NPC, pl.D), dtype=BF16)
    xo[:nreal] = x_pad_bf16[lo_node:hi_node]
    m1 = np.zeros((NPC, pl.D), dtype=np.float32)
    m2 = np.zeros((NPC, pl.D), dtype=np.float32)
    m1[:nreal] = scale1[lo_node:hi_node]
    m2[:nreal] = scale2[lo_node:hi_node]

    return {
        "s_mat": S,
        "idx_all": idx_all,
        "xT": _featT(xo, pl.KB).astype(BF16),
        "m1T": _featT(m1, pl.KB).astype(BF16),
        "m2T": _featT(m2, pl.KB).astype(BF16),
    }


# ----------------------------------------------------------------------------
# Bass program
# ----------------------------------------------------------------------------

def build_program(pl, n_cores=NCORES, g_bufs=3):
    D, DO, KB, NB, NPC, NPAD = pl.D, pl.DO, pl.KB, pl.NB, pl.NPC, pl.NPAD
    NCHUNK, GMAX, HALF = pl.NCHUNK, pl.GMAX, pl.HALF
    Tanh = mybir.ActivationFunctionType.Tanh

    nc = bacc.Bacc("TRN2", target_bir_lowering=False, debug=False,
                   num_devices=n_cores, dynamic_dma_scratch_size=16384,
                   num_swdge_queues=4)

    # ---- DRAM parameters
    x_rows = nc.dram_tensor("x_rows", [NPAD, D], BF16_T, kind="ExternalInput")
    xT_d = nc.dram_tensor("xT", [P, KB, NPC], BF16_T, kind="ExternalInput")
    m1_d = nc.dram_tensor("m1T", [P, KB, NPC], BF16_T, kind="ExternalInput")
    m2_d = nc.dram_tensor("m2T", [P, KB, NPC], BF16_T, kind="ExternalInput")
    s_d = nc.dram_tensor("s_mat", [P, NCHUNK, P], BF16_T, kind="ExternalInput")
    idx_d = nc.dram_tensor("idx_all", [P, NCHUNK * 8], I16, kind="ExternalInput")
    w1l_d = nc.dram_tensor("w1l", [P, KB, D], BF16_T, kind="ExternalInput")
    w1r_d = nc.dram_tensor("w1r", [P, KB, D], BF16_T, kind="ExternalInput")
    w2l_d = nc.dram_tensor("w2l", [P, KB, D], BF16_T, kind="ExternalInput")
    w2r_d = nc.dram_tensor("w2r", [P, KB, D], BF16_T, kind="ExternalInput")
    w3l_d = nc.dram_tensor("w3l", [P, KB, DO], BF16_T, kind="ExternalInput")
    w3r_d = nc.dram_tensor("w3r", [P, KB, DO], BF16_T, kind="ExternalInput")
    bias_d = nc.dram_tensor("bias", [P, 2 * KB + 1], F32, kind="ExternalInput")
    out_d = nc.dram_tensor("out", [NPC, DO], F32, kind="ExternalOutput")

    rg = [list(range(n_cores))]

    with tile.TileContext(nc) as tc, ExitStack() as ctx:
        consts = ctx.enter_context(tc.tile_pool(name="consts", bufs=1))
        gpool = ctx.enter_context(tc.tile_pool(name="gpool", bufs=g_bufs))
        work = ctx.enter_context(tc.tile_pool(name="work", bufs=3))
        dram = ctx.enter_context(tc.tile_pool(name="dram", bufs=1, space="DRAM"))
        psA = ctx.enter_context(tc.tile_pool(name="psA", bufs=2, space="PSUM"))
        psB = ctx.enter_context(tc.tile_pool(name="psB", bufs=2, space="PSUM"))
        psT = ctx.enter_context(tc.tile_pool(name="psT", bufs=2, space="PSUM"))
        psP = ctx.enter_context(tc.tile_pool(name="psP", bufs=2, space="PSUM"))

        # ---- internal DRAM (collective bounce + gathered features)
        cc1_in = dram.tile([NPC, D], BF16_T)
        h1_full = dram.tile([NPAD, D], BF16_T, addr_space="Shared")
        ccp_in = dram.tile([NPC, DO], BF16_T)
        p_full = dram.tile([NPAD, DO], BF16_T, addr_space="Shared")

        # ---- resident SBUF
        s_sb = consts.tile([P, NCHUNK, P], BF16_T)
        nc.sync.dma_start(s_sb[:], s_d[:])
        w1l = consts.tile([P, KB, D], BF16_T)
        nc.sync.dma_start(w1l[:], w1l_d[:])
        w1r = consts.tile([P, KB, D], BF16_T)
        nc.sync.dma_start(w1r[:], w1r_d[:])
        w2l = consts.tile([P, KB, D], BF16_T)
        nc.sync.dma_start(w2l[:], w2l_d[:])
        w2r = consts.tile([P, KB, D], BF16_T)
        nc.sync.dma_start(w2r[:], w2r_d[:])
        w3l = consts.tile([P, KB, DO], BF16_T)
        nc.sync.dma_start(w3l[:], w3l_d[:])
        w3r = consts.tile([P, KB, DO], BF16_T)
        nc.sync.dma_start(w3r[:], w3r_d[:])
        bias_sb = consts.tile([P, 2 * KB + 1], F32)
        nc.sync.dma_start(bias_sb[:], bias_d[:])
        ident_b = consts.tile([P, P], BF16_T)
        make_identity(nc, ident_b)
        ident_f = consts.tile([P, P], F32)
        make_identity(nc, ident_f)

        # per-block resident hT tiles (feature-major current features)
        ht = []
        for b in range(NB):
            t = consts.tile([P, KB, P], BF16_T, name=f"ht{b}", tag=f"ht{b}")
            nc.sync.dma_start(t[:], xT_d[:, :, b * P:(b + 1) * P])
            ht.append(t)

        layers = [
            dict(wl=w1l, wr=w1r, mask=m1_d, bcol=0, src=x_rows, elem=D),
            dict(wl=w2l, wr=w2r, mask=m2_d, bcol=KB, src=h1_full, elem=D),
            dict(wr3=w3r, bcol=2 * KB, src=p_full, elem=DO),
        ]

        gq = [0]

        for li, L in enumerate(layers):
            src_t = L["src"]
            elem = L["elem"]
            src_lo = src_t[0:HALF, :]
            src_hi = src_t[HALF:NPAD, :]
            last3 = li == 2

            for si, sg in enumerate(pl.sgs):
                # gathers for this supergroup
                tiles = {}
                for half, sv in (("lo", src_lo), ("hi", src_hi)):
                    n = sg[half + "_n"]
                    if n == 0:
                        tiles[half] = None
                    else:
                        s0 = sg[half + "_start"]
                        idx_t = work.tile([P, GMAX * 8], I16, tag="idx")
                        nc.sync.dma_start(
                            idx_t[:, :n * 8], idx_d[:, s0 * 8:(s0 + n) * 8])
                        g_t = gpool.tile([P, GMAX, D], BF16_T, tag="g")
                        if last3:
                            gv = g_t.rearrange("p g (a b) -> p (g a) b", b=DO)
                        else:
                            gv = g_t
                        nc.gpsimd.dma_gather(
                            gv[:, :n, :elem], sv, idx_t[:, :n * 8],
                            n * P, n * P, elem, single_packet=False,
                            queue_num=gq[0] % 4)
                        gq[0] += 1
                        tiles[half] = gv

                for b in sg["blocks"]:
                    bsl = slice(b * P, (b + 1) * P)
                    chunks = pl.block_chunks[b]  # (sg_idx, half, slot, cg)
                    my = [(tiles[h], sl, cg) for (s, h, sl, cg) in chunks
                          if s == si]
                    nch = len(my)

                    if not last3:
                        # segment mean (transposed): meanT[feat,dst]
                        mps = psA.tile([P, KB, P], F32, tag="acc")
                        for k in range(KB):
                            for ci, (gt, sl, cg) in enumerate(my):
                                nc.tensor.matmul(
                                    mps[:, k, :],
                                    gt[:, sl, k * P:(k + 1) * P],
                                    s_sb[:, cg, :],
                                    start=(ci == 0), stop=(ci == nch - 1))
                        m_sb = work.tile([P, KB, P], BF16_T, tag="msb")
                        if nch == 0:
                            nc.vector.memset(m_sb[:], 0.0)
                        else:
                            nc.vector.tensor_copy(m_sb[:], mps[:])

                        # dense: outT[feat_out, node] = Wl@meanT + Wr@hT
                        ops = psB.tile([P, KB, P], F32, tag="out")
                        for bank in range(KB):
                            for k in range(KB):
                                nc.tensor.matmul(
                                    ops[:, bank, :],
                                    L["wl"][:, k, bank * P:(bank + 1) * P],
                                    m_sb[:, k, :],
                                    start=(k == 0), stop=False)
                            for k in range(KB):
                                nc.tensor.matmul(
                                    ops[:, bank, :],
                                    L["wr"][:, k, bank * P:(bank + 1) * P],
                                    ht[b][:, k, :],
                                    start=False, stop=(k == KB - 1))

                        # epilogue: tanh(+bias), dropout mask, update hT
                        mk_t = work.tile([P, KB, P], BF16_T, tag="mk")
                        nc.sync.dma_start(mk_t[:], L["mask"][:, :, bsl])
                        a_sb = work.tile([P, KB, P], BF16_T, tag="act")
                        for bank in range(KB):
                            nc.scalar.activation(
                                a_sb[:, bank, :], ops[:, bank, :], Tanh,
                                bias=bias_sb[:, L["bcol"] + bank:L["bcol"] + bank + 1])
                        nc.vector.tensor_mul(
                            out=ht[b][:], in0=a_sb[:], in1=mk_t[:])

                        if li == 0:
                            # node-major copy for AllGather input
                            nm = work.tile([P, D], BF16_T, tag="nm")
                            for bank in range(KB):
                                tp = psT.tile([P, P], BF16_T, tag="tp")
                                nc.tensor.transpose(
                                    tp, ht[b][:, bank, :], ident_b)
                                nc.vector.tensor_copy(
                                    nm[:, bank * P:(bank + 1) * P], tp)
                            nc.sync.dma_start(cc1_in[bsl, :], nm)

                        if li == 1:
                            # p = h2 @ W3l^T  (node-major), for layer-3 gather
                            pp = psP.tile([P, DO], F32, tag="pp")
                            for k in range(KB):
                                nc.tensor.matmul(
                                    pp, ht[b][:, k, :], w3l[:, k, :],
                                    start=(k == 0), stop=(k == KB - 1))
                            p_sb = work.tile([P, DO], BF16_T, tag="pnm")
                            nc.vector.tensor_copy(p_sb, pp)
                            nc.sync.dma_start(ccp_in[bsl, :], p_sb)
                    else:
                        # layer 3: outT = mean(p)^T + W3r @ hT, tanh, output
                        ops = psB.tile([P, KB, P], F32, tag="out")
                        o3 = ops[:, 0, :]
                        for ci, (gt, sl, cg) in enumerate(my):
                            nc.tensor.matmul(
                                o3, gt[:, sl, :], s_sb[:, cg, :],
                                start=(ci == 0), stop=False)
                        for k in range(KB):
                            nc.tensor.matmul(
                                o3, L["wr3"][:, k, :], ht[b][:, k, :],
                                start=(nch == 0 and k == 0), stop=(k == KB - 1))
                        o_sb = work.tile([P, DO], F32, tag="o3")
                        nc.scalar.activation(
                            o_sb, o3, Tanh,
                            bias=bias_sb[:, L["bcol"]:L["bcol"] + 1])
                        tpf = psP.tile([P, DO], F32, tag="pp")
                        nc.tensor.transpose(tpf, o_sb, ident_f)
                        onm = work.tile([P, DO], F32, tag="onm")
                        nc.vector.tensor_copy(onm, tpf)
                        nc.sync.dma_start(out_d[bsl, :], onm)

            # collectives at layer boundaries
            if li == 0:
                nc.gpsimd.collective_compute(
                    "AllGather", mybir.AluOpType.bypass,
                    replica_groups=rg,
                    ins=[cc1_in.opt()], outs=[h1_full.opt()])
            elif li == 1:
                nc.gpsimd.collective_compute(
                    "AllGather", mybir.AluOpType.bypass,
                    replica_groups=rg,
                    ins=[ccp_in.opt()], outs=[p_full.opt()])

    nc.compile()
    return nc


# ----------------------------------------------------------------------------
# Host driver
# ----------------------------------------------------------------------------

def prepare(x, edge_index, mask1, mask2,
            W1l, b1, W1r, W2l, b2, W2r, W3l, b3, W3r, gmax=22):
    N, D = x.shape
    DO = W3l.shape[0]
    E = edge_index.shape[1]
    pl = make_plan(N, E, D, DO, edge_index, gmax=gmax)
    KB = pl.KB

    x_bf = x.astype(BF16)
    x_pad = np.zeros((pl.NPAD, D), dtype=BF16)
    x_pad[:N] = x_bf
    scale1 = ((mask1 > DROP_P) / (1.0 - DROP_P)).astype(np.float32)
    scale2 = ((mask2 > DROP_P) / (1.0 - DROP_P)).astype(np.float32)

    def packw(W):
        return np.ascontiguousarray(
            W.T.reshape(KB, P, W.shape[0]).transpose(1, 0, 2)).astype(BF16)

    bias = np.zeros((P, 2 * KB + 1), dtype=np.float32)
    for k in range(KB):
        bias[:, k] = b1[k * P:(k + 1) * P]
        bias[:, KB + k] = b2[k * P:(k + 1) * P]
    bias[:, 2 * KB] = b3[:P]

    shared = {
        "x_rows": x_pad,
        "w1l": packw(W1l), "w1r": packw(W1r),
        "w2l": packw(W2l), "w2r": packw(W2r),
        "w3l": packw(W3l), "w3r": packw(W3r),
        "bias": bias,
    }
    in_maps = []
    for c in range(NCORES):
        m = dict(shared)
        m.update(pack_core(pl, c, x_pad, scale1, scale2))
        in_maps.append(m)
    return pl, in_maps


_CACHE = {}


def kernel(x, edge_index, mask1, mask2,
           W1l, b1, W1r, W2l, b2, W2r, W3l, b3, W3r):
    x = np.asarray(x, dtype=np.float32)
    args = (np.asarray(edge_index), np.asarray(mask1, dtype=np.float32),
            np.asarray(mask2, dtype=np.float32))
    pl, in_maps = prepare(
        x, args[0], args[1], args[2],
        np.asarray(W1l, np.float32), np.asarray(b1, np.float32),
        np.asarray(W1r, np.float32),
        np.asarray(W2l, np.float32), np.asarray(b2, np.float32),
        np.asarray(W2r, np.float32),
        np.asarray(W3l, np.float32), np.asarray(b3, np.float32),
        np.asarray(W3r, np.float32))
    nc = build_program(pl)
    res = run_bass_kernel_spmd(nc, in_maps, core_ids=list(range(NCORES)))
    N = x.shape[0]
    out = np.zeros((N, pl.DO), dtype=np.float32)
    for c in range(NCORES):
        lo, hi = c * pl.NPC, min((c + 1) * pl.NPC, N)
        out[lo:hi] = res.results[c]["out"][:hi - lo]
    return out


# revision 12
# speedup vs baseline: 1.6253x; 1.5695x over previous
"""GraphSAGE 3-layer GNN on 8 TRN2 NeuronCores.

Strategy (node-sharded, feature-replicated):
  - 50000 nodes padded to 50176 = 8 cores x 49 blocks x 128. Core c owns
    destination rows [6272c, 6272c+6272).
  - Per layer, every core gathers h[src] rows for its local edges with
    gpsimd.dma_gather (one 512B/256B row per edge, edge lands on one SBUF
    partition), then segment-sums them per 128-dst block with a one-hot
    (x 1/deg) matmul on TensorE:  meanT[feat,dst] += G[e,feat]^T.T @ S[e,dst].
  - Dense SAGE terms run in transposed orientation so tanh bias is
    per-partition:  outT[feat_out, node] = Wl @ meanT + Wr @ hT (+bias).
  - Dropout masks are precomputed on the host as {0, 1.25} multipliers.
  - Between layers the node-major output shard is AllGather'd so every core
    can gather any source row next layer. Layer 3 aggregates the
    W3l-projected features p = h2 @ W3l^T (128 dims), so only p is gathered
    (halves layer-3 traffic); h2 stays local for the W3r term.
  - int16 gather indices can't span 50176 rows, so the node space is split
    at HALF: per (block, half) edge chunks gather from the lo/hi view.

kernel(**inputs) takes full-size numpy inputs, does all index/layout prep on
the host, compiles one SPMD Bass program, runs it on cores 0-7, and returns
the full [50000, 128] float32 output.
"""

import math
from contextlib import ExitStack

import ml_dtypes
import numpy as np

import concourse.bass as bass
import concourse.bacc as bacc
import concourse.mybir as mybir
import concourse.tile as tile
from concourse.bass_utils import run_bass_kernel_spmd
from concourse.masks import make_identity

P = 128
NCORES = 8
DROP_P = 0.2
BF16 = ml_dtypes.bfloat16

F32 = mybir.dt.float32
BF16_T = mybir.dt.bfloat16
I16 = mybir.dt.int16


# ----------------------------------------------------------------------------
# Planning (shared across cores; the SPMD program structure depends on it)
# ----------------------------------------------------------------------------

class Plan:
    pass


def make_plan(n_nodes, n_edges, d_in, d_out, edge_index, gmax=22):
    pl = Plan()
    pl.N = n_nodes
    pl.E = n_edges
    pl.D = d_in
    pl.DO = d_out
    pl.KB = d_in // P
    assert d_in % P == 0 and d_out == P
    pl.NB = math.ceil(n_nodes / NCORES / P)     # blocks per core
    pl.NPC = pl.NB * P                          # padded nodes per core
    pl.NPAD = NCORES * pl.NPC

    src = np.asarray(edge_index[0], dtype=np.int64)
    dst = np.asarray(edge_index[1], dtype=np.int64)
    assert src.min() >= 0 and src.max() < n_nodes
    deg = np.bincount(dst, minlength=n_nodes)
    pl.invdeg = (1.0 / np.maximum(deg, 1)).astype(np.float32)

    core = dst // pl.NPC
    lb = (dst - core * pl.NPC) // P             # block within core
    pl.dst_local = (dst - core * pl.NPC) % P
    pl.core = core
    pl.lb = lb
    pl.src = src
    pl.dst = dst

    # pick HALF (node-id split so gather indices fit int16) minimizing chunks
    lo_min = max(0, pl.NPAD - 32768)
    candidates = list(range(max(1024, lo_min), min(32768, pl.NPAD) + 1, 1024))
    if pl.NPAD <= 32768:
        candidates = [pl.NPAD]  # no split needed
    best = None
    key_base = (core * pl.NB + lb) * 2
    for half in candidates:
        is_hi = (src >= half).astype(np.int64)
        cnt = np.bincount(key_base + is_hi, minlength=NCORES * pl.NB * 2)
        cnt = cnt.reshape(NCORES, pl.NB, 2)
        nchunks = -(-cnt.max(axis=0) // P)      # [NB, 2] ceil
        tot = int(nchunks.sum())
        if best is None or tot < best[0]:
            best = (tot, half, nchunks)
    pl.total_chunks_min, pl.HALF, nch = best
    pl.nlo = nch[:, 0].astype(int)
    pl.nhi = nch[:, 1].astype(int)

    # supergroups: consecutive blocks with per-half chunk sums <= gmax
    sgs = []
    cur, slo, shi = [], 0, 0
    for b in range(pl.NB):
        if cur and (slo + pl.nlo[b] > gmax or shi + pl.nhi[b] > gmax):
            sgs.append(cur)
            cur, slo, shi = [], 0, 0
        cur.append(b)
        slo += pl.nlo[b]
        shi += pl.nhi[b]
    if cur:
        sgs.append(cur)

    # global chunk ordering: per sg -> lo chunks (block order) -> hi chunks
    pl.sgs = []
    cg = 0
    pl.block_chunks = [None] * pl.NB   # block -> list of (sg_idx, half, local_slot, cg)
    for b in range(pl.NB):
        pl.block_chunks[b] = []
    for si, blocks in enumerate(sgs):
        info = {"blocks": blocks}
        for half, narr in (("lo", pl.nlo), ("hi", pl.nhi)):
            start = cg
            for b in blocks:
                for j in range(narr[b]):
                    pl.block_chunks[b].append((si, half, cg - start, cg))
                    cg += 1
            info[half + "_start"] = start
            info[half + "_n"] = cg - start
        pl.sgs.append(info)
    pl.NCHUNK = cg
    pl.GMAX = max(
        max((sg["lo_n"] for sg in pl.sgs), default=1),
        max((sg["hi_n"] for sg in pl.sgs), default=1),
        1,
    )
    return pl


# ----------------------------------------------------------------------------
# Per-core host packing
# ----------------------------------------------------------------------------

def _featT(arr, KB):
    """[n, KB*128] -> [128, KB, n]  (feature-major layout)."""
    n = arr.shape[0]
    return np.ascontiguousarray(
        arr.T.reshape(KB, P, n).transpose(1, 0, 2)
    )


def pack_core(pl, c, x_pad_bf16, scale1, scale2):
    """Build S, idx, xT, maskT arrays for core c."""
    NB, NPC, NCHUNK = pl.NB, pl.NPC, pl.NCHUNK
    mine = pl.core == c
    src, lb, dl = pl.src[mine], pl.lb[mine], pl.dst_local[mine]
    w = pl.invdeg[pl.dst[mine]]
    is_hi = src >= pl.HALF

    # slot assignment: order edges by (lb, half); chunk starts from plan
    order = np.lexsort((is_hi, lb))
    src, lb, dl, w, is_hi = (a[order] for a in (src, lb, dl, w, is_hi))

    # chunk start (in global cg units) per (block, half)
    cg_start = np.zeros((NB, 2), dtype=np.int64)
    for b in range(NB):
        lo_list = [cg for (_, h, _, cg) in pl.block_chunks[b] if h == "lo"]
        hi_list = [cg for (_, h, _, cg) in pl.block_chunks[b] if h == "hi"]
        cg_start[b, 0] = lo_list[0] if lo_list else 0
        cg_start[b, 1] = hi_list[0] if hi_list else 0

    # rank within (block, half) group
    key = lb * 2 + is_hi
    grp_first = np.searchsorted(key, np.arange(NB * 2))
    rank = np.arange(len(src)) - grp_first[key]
    pos = cg_start[lb, is_hi.astype(int)] * P + rank           # global slot
    cap = None  # sanity: rank must fit the planned chunks
    nch = np.stack([pl.nlo, pl.nhi], axis=1)
    assert (rank < nch[lb, is_hi.astype(int)] * P).all(), "chunk overflow"

    idx_vals = np.zeros(NCHUNK * P, dtype=np.int16)
    idx_vals[pos] = np.where(is_hi, src - pl.HALF, src).astype(np.int16)

    S = np.zeros((P, NCHUNK, P), dtype=BF16)
    S[pos % P, pos // P, dl] = w.astype(BF16)

    # wrap idx into [128, NCHUNK*8] int16, per-gather-call 16-element wrap
    idx_all = np.zeros((P, NCHUNK * 8), dtype=np.int16)
    for sg in pl.sgs:
        for half in ("lo", "hi"):
            s0, n = sg[half + "_start"], sg[half + "_n"]
            if n == 0:
                continue
            seg = idx_vals[s0 * P:(s0 + n) * P]
            blk = seg.reshape(-1, 16).T                        # [16, n*8]
            idx_all[:, s0 * 8:(s0 + n) * 8] = np.tile(blk, (8, 1))

    lo_node, hi_node = c * NPC, min((c + 1) * NPC, pl.N)
    nreal = hi_node - lo_node
    xo = np.zeros((NPC, pl.D), dtype=BF16)
    xo[:nreal] = x_pad_bf16[lo_node:hi_node]
    m1 = np.zeros((NPC, pl.D), dtype=np.float32)
    m2 = np.zeros((NPC, pl.D), dtype=np.float32)
    m1[:nreal] = scale1[lo_node:hi_node]
    m2[:nreal] = scale2[lo_node:hi_node]

    return {
        "s_mat": S,
        "idx_all": idx_all,
        "xT": _featT(xo, pl.KB).astype(BF16),
        "m1T": _featT(m1, pl.KB).astype(BF16),
        "m2T": _featT(m2, pl.KB).astype(BF16),
    }


# ----------------------------------------------------------------------------
# Bass program
# ----------------------------------------------------------------------------

def build_program(pl, n_cores=NCORES, g_bufs=3):
    D, DO, KB, NB, NPC, NPAD = pl.D, pl.DO, pl.KB, pl.NB, pl.NPC, pl.NPAD
    NCHUNK, GMAX, HALF = pl.NCHUNK, pl.GMAX, pl.HALF
    Tanh = mybir.ActivationFunctionType.Tanh

    nc = bacc.Bacc("TRN2", target_bir_lowering=False, debug=False,
                   num_devices=n_cores, dynamic_dma_scratch_size=16384,
                   num_swdge_queues=4)

    # ---- DRAM parameters
    x_rows = nc.dram_tensor("x_rows", [NPAD, D], BF16_T, kind="ExternalInput")
    xT_d = nc.dram_tensor("xT", [P, KB, NPC], BF16_T, kind="ExternalInput")
    m1_d = nc.dram_tensor("m1T", [P, KB, NPC], BF16_T, kind="ExternalInput")
    m2_d = nc.dram_tensor("m2T", [P, KB, NPC], BF16_T, kind="ExternalInput")
    s_d = nc.dram_tensor("s_mat", [P, NCHUNK, P], BF16_T, kind="ExternalInput")
    idx_d = nc.dram_tensor("idx_all", [P, NCHUNK * 8], I16, kind="ExternalInput")
    w1l_d = nc.dram_tensor("w1l", [P, KB, D], BF16_T, kind="ExternalInput")
    w1r_d = nc.dram_tensor("w1r", [P, KB, D], BF16_T, kind="ExternalInput")
    w2l_d = nc.dram_tensor("w2l", [P, KB, D], BF16_T, kind="ExternalInput")
    w2r_d = nc.dram_tensor("w2r", [P, KB, D], BF16_T, kind="ExternalInput")
    w3l_d = nc.dram_tensor("w3l", [P, KB, DO], BF16_T, kind="ExternalInput")
    w3r_d = nc.dram_tensor("w3r", [P, KB, DO], BF16_T, kind="ExternalInput")
    bias_d = nc.dram_tensor("bias", [P, 2 * KB + 1], F32, kind="ExternalInput")
    out_d = nc.dram_tensor("out", [NPC, DO], F32, kind="ExternalOutput")

    rg = [list(range(n_cores))]

    with tile.TileContext(nc) as tc, ExitStack() as ctx:
        consts = ctx.enter_context(tc.tile_pool(name="consts", bufs=1))
        gpool = ctx.enter_context(tc.tile_pool(name="gpool", bufs=g_bufs))
        work = ctx.enter_context(tc.tile_pool(name="work", bufs=3))
        dram = ctx.enter_context(tc.tile_pool(name="dram", bufs=1, space="DRAM"))
        psA = ctx.enter_context(tc.tile_pool(name="psA", bufs=2, space="PSUM"))
        psB = ctx.enter_context(tc.tile_pool(name="psB", bufs=2, space="PSUM"))
        psT = ctx.enter_context(tc.tile_pool(name="psT", bufs=2, space="PSUM"))
        psP = ctx.enter_context(tc.tile_pool(name="psP", bufs=2, space="PSUM"))

        # ---- internal DRAM (collective bounce + gathered features)
        cc1_in = dram.tile([NPC, D], BF16_T)
        h1_full = dram.tile([NPAD, D], BF16_T, addr_space="Shared")
        ccp_in = dram.tile([NPC, DO], BF16_T)
        p_full = dram.tile([NPAD, DO], BF16_T, addr_space="Shared")

        # ---- resident SBUF
        idx_sb = consts.tile([P, NCHUNK * 8], I16)
        nc.sync.dma_start(idx_sb[:], idx_d[:])
        w1l = consts.tile([P, KB, D], BF16_T)
        nc.sync.dma_start(w1l[:], w1l_d[:])
        w1r = consts.tile([P, KB, D], BF16_T)
        nc.sync.dma_start(w1r[:], w1r_d[:])
        w2l = consts.tile([P, KB, D], BF16_T)
        nc.sync.dma_start(w2l[:], w2l_d[:])
        w2r = consts.tile([P, KB, D], BF16_T)
        nc.sync.dma_start(w2r[:], w2r_d[:])
        w3l = consts.tile([P, KB, DO], BF16_T)
        nc.sync.dma_start(w3l[:], w3l_d[:])
        w3r = consts.tile([P, KB, DO], BF16_T)
        nc.sync.dma_start(w3r[:], w3r_d[:])
        bias_sb = consts.tile([P, 2 * KB + 1], F32)
        nc.sync.dma_start(bias_sb[:], bias_d[:])
        ident_b = consts.tile([P, P], BF16_T)
        make_identity(nc, ident_b)
        ident_f = consts.tile([P, P], F32)
        make_identity(nc, ident_f)

        # per-block resident hT tiles (feature-major current features)
        ht = []
        for b in range(NB):
            t = consts.tile([P, KB, P], BF16_T, name=f"ht{b}", tag=f"ht{b}")
            nc.sync.dma_start(t[:], xT_d[:, :, b * P:(b + 1) * P])
            ht.append(t)

        layers = [
            dict(wl=w1l, wr=w1r, mask=m1_d, bcol=0, src=x_rows, elem=D),
            dict(wl=w2l, wr=w2r, mask=m2_d, bcol=KB, src=h1_full, elem=D),
            dict(wr3=w3r, bcol=2 * KB, src=p_full, elem=DO),
        ]

        SMAX = max(sg["lo_n"] + sg["hi_n"] for sg in pl.sgs)
        gq = [0]

        for li, L in enumerate(layers):
            src_t = L["src"]
            elem = L["elem"]
            src_lo = src_t[0:HALF, :]
            src_hi = src_t[HALF:NPAD, :]
            last3 = li == 2

            for si, sg in enumerate(pl.sgs):
                # stream this supergroup's S chunks (contiguous cg range)
                sg_c0 = sg["lo_start"]
                sg_nc = sg["lo_n"] + sg["hi_n"]
                s_t = gpool.tile([P, SMAX, P], BF16_T, tag="s")
                nc.scalar.dma_start(
                    s_t[:, :sg_nc, :], s_d[:, sg_c0:sg_c0 + sg_nc, :])
                # gathers for this supergroup
                tiles = {}
                for half, sv in (("lo", src_lo), ("hi", src_hi)):
                    n = sg[half + "_n"]
                    if n == 0:
                        tiles[half] = None
                    else:
                        s0 = sg[half + "_start"]
                        g_t = gpool.tile([P, GMAX, D], BF16_T, tag="g" + half)
                        if last3:
                            gv = g_t.rearrange("p g (a b) -> p (g a) b", b=DO)
                        else:
                            gv = g_t
                        nc.gpsimd.dma_gather(
                            gv[:, :n, :elem], sv,
                            idx_sb[:, s0 * 8:(s0 + n) * 8],
                            n * P, n * P, elem, single_packet=False,
                            queue_num=gq[0] % 4)
                        gq[0] += 1
                        tiles[half] = gv

                for b in sg["blocks"]:
                    bsl = slice(b * P, (b + 1) * P)
                    chunks = pl.block_chunks[b]  # (sg_idx, half, slot, cg)
                    my = [(tiles[h], sl, cg) for (s, h, sl, cg) in chunks
                          if s == si]
                    nch = len(my)

                    if not last3:
                        # segment mean (transposed): meanT[feat,dst]
                        mps = psA.tile([P, KB, P], F32, tag="acc")
                        for k in range(KB):
                            for ci, (gt, sl, cg) in enumerate(my):
                                nc.tensor.matmul(
                                    mps[:, k, :],
                                    gt[:, sl, k * P:(k + 1) * P],
                                    s_t[:, cg - sg_c0, :],
                                    start=(ci == 0), stop=(ci == nch - 1))
                        m_sb = work.tile([P, KB, P], BF16_T, tag="msb")
                        if nch == 0:
                            nc.vector.memset(m_sb[:], 0.0)
                        else:
                            nc.vector.tensor_copy(m_sb[:], mps[:])

                        # dense: outT[feat_out, node] = Wl@meanT + Wr@hT
                        ops = psB.tile([P, KB, P], F32, tag="out")
                        for bank in range(KB):
                            for k in range(KB):
                                nc.tensor.matmul(
                                    ops[:, bank, :],
                                    L["wl"][:, k, bank * P:(bank + 1) * P],
                                    m_sb[:, k, :],
                                    start=(k == 0), stop=False)
                            for k in range(KB):
                                nc.tensor.matmul(
                                    ops[:, bank, :],
                                    L["wr"][:, k, bank * P:(bank + 1) * P],
                                    ht[b][:, k, :],
                                    start=False, stop=(k == KB - 1))

                        # epilogue: tanh(+bias), dropout mask, update hT
                        mk_t = work.tile([P, KB, P], BF16_T, tag="mk")
                        nc.sync.dma_start(mk_t[:], L["mask"][:, :, bsl])
                        a_sb = work.tile([P, KB, P], BF16_T, tag="act")
                        for bank in range(KB):
                            nc.scalar.activation(
                                a_sb[:, bank, :], ops[:, bank, :], Tanh,
                                bias=bias_sb[:, L["bcol"] + bank:L["bcol"] + bank + 1])
                        nc.vector.tensor_mul(
                            out=ht[b][:], in0=a_sb[:], in1=mk_t[:])

                        if li == 0:
                            # node-major copy for AllGather input
                            nm = work.tile([P, D], BF16_T, tag="nm")
                            for bank in range(KB):
                                tp = psT.tile([P, P], BF16_T, tag="tp")
                                nc.tensor.transpose(
                                    tp, ht[b][:, bank, :], ident_b)
                                nc.vector.tensor_copy(
                                    nm[:, bank * P:(bank + 1) * P], tp)
                            nc.sync.dma_start(cc1_in[bsl, :], nm)

                        if li == 1:
                            # p = h2 @ W3l^T  (node-major), for layer-3 gather
                            pp = psP.tile([P, DO], F32, tag="pp")
                            for k in range(KB):
                                nc.tensor.matmul(
                                    pp, ht[b][:, k, :], w3l[:, k, :],
                                    start=(k == 0), stop=(k == KB - 1))
                            p_sb = work.tile([P, DO], BF16_T, tag="pnm")
                            nc.vector.tensor_copy(p_sb, pp)
                            nc.sync.dma_start(ccp_in[bsl, :], p_sb)
                    else:
                        # layer 3: outT = mean(p)^T + W3r @ hT, tanh, output
                        ops = psB.tile([P, KB, P], F32, tag="out")
                        o3 = ops[:, 0, :]
                        for ci, (gt, sl, cg) in enumerate(my):
                            nc.tensor.matmul(
                                o3, gt[:, sl, :], s_t[:, cg - sg_c0, :],
                                start=(ci == 0), stop=False)
                        for k in range(KB):
                            nc.tensor.matmul(
                                o3, L["wr3"][:, k, :], ht[b][:, k, :],
                                start=(nch == 0 and k == 0), stop=(k == KB - 1))
                        o_sb = work.tile([P, DO], F32, tag="o3")
                        nc.scalar.activation(
                            o_sb, o3, Tanh,
                            bias=bias_sb[:, L["bcol"]:L["bcol"] + 1])
                        tpf = psP.tile([P, DO], F32, tag="pp")
                        nc.tensor.transpose(tpf, o_sb, ident_f)
                        onm = work.tile([P, DO], F32, tag="onm")
                        nc.vector.tensor_copy(onm, tpf)
                        nc.sync.dma_start(out_d[bsl, :], onm)

            # collectives at layer boundaries
            if li == 0:
                nc.gpsimd.collective_compute(
                    "AllGather", mybir.AluOpType.bypass,
                    replica_groups=rg,
                    ins=[cc1_in.opt()], outs=[h1_full.opt()])
            elif li == 1:
                nc.gpsimd.collective_compute(
                    "AllGather", mybir.AluOpType.bypass,
                    replica_groups=rg,
                    ins=[ccp_in.opt()], outs=[p_full.opt()])

    nc.compile()
    return nc


# ----------------------------------------------------------------------------
# Host driver
# ----------------------------------------------------------------------------

def prepare(x, edge_index, mask1, mask2,
            W1l, b1, W1r, W2l, b2, W2r, W3l, b3, W3r, gmax=22):
    N, D = x.shape
    DO = W3l.shape[0]
    E = edge_index.shape[1]
    pl = make_plan(N, E, D, DO, edge_index, gmax=gmax)
    KB = pl.KB

    x_bf = x.astype(BF16)
    x_pad = np.zeros((pl.NPAD, D), dtype=BF16)
    x_pad[:N] = x_bf
    scale1 = ((mask1 > DROP_P) / (1.0 - DROP_P)).astype(np.float32)
    scale2 = ((mask2 > DROP_P) / (1.0 - DROP_P)).astype(np.float32)

    def packw(W):
        return np.ascontiguousarray(
            W.T.reshape(KB, P, W.shape[0]).transpose(1, 0, 2)).astype(BF16)

    bias = np.zeros((P, 2 * KB + 1), dtype=np.float32)
    for k in range(KB):
        bias[:, k] = b1[k * P:(k + 1) * P]
        bias[:, KB + k] = b2[k * P:(k + 1) * P]
    bias[:, 2 * KB] = b3[:P]

    shared = {
        "x_rows": x_pad,
        "w1l": packw(W1l), "w1r": packw(W1r),
        "w2l": packw(W2l), "w2r": packw(W2r),
        "w3l": packw(W3l), "w3r": packw(W3r),
        "bias": bias,
    }
    in_maps = []
    for c in range(NCORES):
        m = dict(shared)
        m.update(pack_core(pl, c, x_pad, scale1, scale2))
        in_maps.append(m)
    return pl, in_maps


_CACHE = {}


def kernel(x, edge_index, mask1, mask2,
           W1l, b1, W1r, W2l, b2, W2r, W3l, b3, W3r):
    x = np.asarray(x, dtype=np.float32)
    args = (np.asarray(edge_index), np.asarray(mask1, dtype=np.float32),
            np.asarray(mask2, dtype=np.float32))
    pl, in_maps = prepare(
        x, args[0], args[1], args[2],
        np.asarray(W1l, np.float32), np.asarray(b1, np.float32),
        np.asarray(W1r, np.float32),
        np.asarray(W2l, np.float32), np.asarray(b2, np.float32),
        np.asarray(W2r, np.float32),
        np.asarray(W3l, np.float32), np.asarray(b3, np.float32),
        np.asarray(W3r, np.float32))
    nc = build_program(pl)
    res = run_bass_kernel_spmd(nc, in_maps, core_ids=list(range(NCORES)))
    N = x.shape[0]
    out = np.zeros((N, pl.DO), dtype=np.float32)
    for c in range(NCORES):
        lo, hi = c * pl.NPC, min((c + 1) * pl.NPC, N)
        out[lo:hi] = res.results[c]["out"][:hi - lo]
    return out
